# revision 10
# baseline (speedup 1.0000x reference)
"""Pairwise cosine similarity  O = (Z/|Z_rows|) @ (Y/|Y_rows|).T  on 8 TRN2 cores.

Sharding: Z rows split across 8 cores (data parallel), Y replicated.
Each core computes O^T block [4096, 512] (y-major); host transposes back.

v7 structure:
  - inputs are loaded fp32->bf16 by casting gpsimd (SWDGE) DMAs; the whole
    on-chip pipeline is bf16; fp32 PSUM accumulation keeps dots accurate.
  - ALL transposes run on the DMA XBAR (InstDmaTransposeAnt, 16-bit
    SBUF->SBUF): one instruction per 128-row tile transposes [128, 4096]
    straight into the k-sliced operand layout ([kk, k, row] 3D out AP).
    The PE runs nothing but matmuls; PSUM transpose staging and the DVE
    copybacks are gone.
  - matmul orientation: transposed-Y tiles are the STATIONARY operand
    [128k, 128y]; the SBUF-resident Z^T cache (kxm, bf16) is the MOVING
    operand [128k, 512z] (1 cyc/row, N=512). Output blocks are
    [y-part, z-free] so 1/|y| is a per-partition activation scale on the
    scalar engine; the output is O^T, un-transposed on the host.
  - chunk-level software pipeline (lag 1): XBAR transposes chunk c while
    the PE multiplies chunk c-1. kxm bufs=2 overlaps the next bench
    iteration's Z phase with this iteration's tail.
  - row sumsq = one Square activation with accum_out per 128-row tile.
  - queues: inputs on gpsimd (SWDGE, casting), Y/Z transposes on the
    sync HWDGE queue, outputs on the Activation HWDGE queue.
"""

import contextlib
import os
import sys
import numpy as np

_TRN_REPO = "/opt/trn_rl_repo"
if _TRN_REPO not in sys.path:
    sys.path.insert(0, _TRN_REPO)

import concourse.bacc as bacc
import concourse.mybir as mybir
import concourse.tile as tile
from concourse.bass_utils import run_bass_kernel_spmd

P = 128
N_CORES = 8
F32 = mybir.dt.float32
BF16 = mybir.dt.bfloat16


def build(bz_core=512, by=4096, feat=4096, n_chunk=256, bench_iters=None):
    """Build + bacc-compile the SPMD program (same program on every core)."""
    assert bz_core % P == 0 and by % n_chunk == 0 and feat % P == 0
    m_sub = bz_core // P          # z sub-tiles in the kxm free dim
    k_tiles = feat // P           # contraction tiles
    n_chunks = by // n_chunk      # Y row chunks
    j_sub = n_chunk // P          # y sub-tiles per chunk (= acc banks)

    nc = bacc.Bacc("TRN2", target_bir_lowering=False, debug=False,
                   num_devices=N_CORES)
    if bench_iters is None:
        z = nc.dram_tensor("z", [bz_core, feat], F32, kind="ExternalInput").ap()
        y = nc.dram_tensor("y", [by, feat], F32, kind="ExternalInput").ap()
        # o holds this core's O^T block [by, bz_core]
        o = nc.dram_tensor("o", [by, bz_core], F32, kind="ExternalOutput").ap()
    else:
        # bench mode: no host I/O, garbage-content internal tensors
        z = nc.dram_tensor("zi", [bz_core, feat], F32).ap()
        y = nc.dram_tensor("yi", [by, feat], F32).ap()
        o = nc.dram_tensor("oi", [by, bz_core], F32).ap()
        dummy_in = nc.dram_tensor("dummy_in", [1, 64], F32,
                                  kind="ExternalInput").ap()
        dummy_out = nc.dram_tensor("dummy_out", [1, 64], F32,
                                   kind="ExternalOutput").ap()

    with tile.TileContext(nc) as tc:
        with tc.tile_pool(name="kxm", bufs=2) as kxm_pool, \
             tc.tile_pool(name="nat", bufs=3) as nat_pool, \
             tc.tile_pool(name="small", bufs=2) as small_pool, \
             tc.tile_pool(name="sq", bufs=2) as sq_pool, \
             tc.tile_pool(name="yt", bufs=2) as yt_pool, \
             tc.tile_pool(name="outs", bufs=3) as out_pool, \
             tc.tile_pool(name="pacc", bufs=2, space="PSUM") as pacc_pool:

            if bench_iters is None:
                _loop = contextlib.nullcontext()
            else:
                _loop = tc.For_i(0, bench_iters, 1)
            with _loop:
                def row_rnorm(nat_ap, rdst):
                    """rdst[p,0] = 1/|row p| for a [P, feat] natural tile.

                    One Square activation with accum_out = full row sumsq;
                    the bf16 elementwise output is scratch (values unused).
                    """
                    sq = sq_pool.tile([P, feat], BF16, tag="sqscratch")
                    ss = small_pool.tile([P, 1], F32, tag="ss")
                    nc.scalar.activation(
                        sq[:], nat_ap,
                        mybir.ActivationFunctionType.Square,
                        accum_out=ss[:])
                    std = small_pool.tile([P, 1], F32, tag="std")
                    nc.scalar.sqrt(std[:], ss[:])
                    nc.vector.reciprocal(rdst, std[:])

                # ---- Z phase: norms + prescale + XBAR-transpose into kxm ----
                # Z is loaded as ynat-shaped halves so the nat pool slots all
                # have one size and Z buffers recycle into Y chunk buffers.
                assert m_sub % j_sub == 0
                zn_tiles = []
                for h in range(m_sub // j_sub):
                    znh = nat_pool.tile([P, j_sub, feat], BF16, tag="nat",
                                        name=f"zn{h}")
                    for jj in range(j_sub):
                        j = h * j_sub + jj
                        nc.gpsimd.dma_start(out=znh[:, jj],
                                            in_=z[j * P:(j + 1) * P, :])
                    zn_tiles.append(znh)
                rz = small_pool.tile([P, m_sub], F32, tag="rz")
                for j in range(m_sub):
                    znj = zn_tiles[j // j_sub][:, j % j_sub]
                    row_rnorm(znj, rz[:, j:j + 1])
                    nc.vector.tensor_scalar_mul(znj, znj, rz[:, j:j + 1])
                # kxm[kk, k, z] = Zn[z, k*128+kk]
                kxm = kxm_pool.tile([P, k_tiles, bz_core], BF16)
                for j in range(m_sub):
                    znj = zn_tiles[j // j_sub][:, j % j_sub]
                    nc.sync.dma_start_transpose(
                        kxm[:, :, j * P:(j + 1) * P], znj)

                # ---- main loop over Y chunks (lag-1 chunk pipeline) ----
                ynats = {}
                rys = {}
                accs = {}
                yts = {}

                def start_chunk(c):
                    ynat = nat_pool.tile([P, j_sub, feat], BF16, tag="nat")
                    for j in range(j_sub):
                        nc.gpsimd.dma_start(
                            out=ynat[:, j],
                            in_=y[c * n_chunk + j * P:
                                  c * n_chunk + (j + 1) * P, :])
                    ry = small_pool.tile([P, j_sub], F32, tag="ry")
                    for j in range(j_sub):
                        row_rnorm(ynat[:, j], ry[:, j:j + 1])
                    # yt[kk, (j k q)] = Yn[c*n_chunk + j*128 + q, k*128 + kk]
                    yt = yt_pool.tile([P, j_sub * k_tiles * P], BF16, tag="yt")
                    for j in range(j_sub):
                        nc.sync.dma_start_transpose(
                            yt[:, j * k_tiles * P:(j + 1) * k_tiles * P]
                            .rearrange("p (k q) -> p k q", k=k_tiles),
                            ynat[:, j])
                    ynats[c] = ynat
                    rys[c] = ry
                    yts[c] = yt
                    accs[c] = [pacc_pool.tile([P, bz_core], F32,
                                              tag=f"acc{j}", name=f"acc{j}")
                               for j in range(j_sub)]

                def emit_matmuls(c):
                    yt = yts.pop(c)
                    del ynats[c]
                    for k in range(k_tiles):
                        for j in range(j_sub):
                            nc.tensor.matmul(
                                accs[c][j][:],
                                yt[:, (j * k_tiles + k) * P:
                                   (j * k_tiles + k + 1) * P],
                                kxm[:, k, :],
                                start=(k == 0),
                                stop=(k == k_tiles - 1))
                    evict_chunk(c)

                def evict_chunk(c):
                    ry = rys.pop(c)
                    for j in range(j_sub):
                        ob = out_pool.tile([P, bz_core], F32, tag="ob")
                        nc.scalar.activation(
                            ob[:], accs[c][j][:],
                            mybir.ActivationFunctionType.Copy,
                            scale=ry[:, j:j + 1])
                        nc.scalar.dma_start(
                            out=o[c * n_chunk + j * P:
                                  c * n_chunk + (j + 1) * P, :],
                            in_=ob[:])
                    del accs[c]

                for c in range(n_chunks + 1):
                    if c < n_chunks:
                        start_chunk(c)
                    if c >= 1:
                        emit_matmuls(c - 1)

            if bench_iters is not None:
                db = small_pool.tile([1, 64], F32, tag="db", name="db")
                nc.sync.dma_start(out=db[:], in_=dummy_in[:])
                nc.vector.tensor_copy(db[:], db[:])
                nc.sync.dma_start(out=dummy_out[:], in_=db[:])

    nc.compile()
    return nc


_CACHE = {}


def _get_compiled():
    if "nc" not in _CACHE:
        _CACHE["nc"] = build()
    return _CACHE["nc"]


def kernel(Z, Y):
    Z = np.ascontiguousarray(np.asarray(Z, dtype=np.float32))
    Y = np.ascontiguousarray(np.asarray(Y, dtype=np.float32))
    bz = Z.shape[0]
    shard = bz // N_CORES
    nc = _get_compiled()
    in_maps = [{"z": Z[i * shard:(i + 1) * shard], "y": Y}
               for i in range(N_CORES)]
    res = run_bass_kernel_spmd(nc, in_maps, list(range(N_CORES)))
    # each core returns O^T block [by, shard]; stitch + transpose back
    out_t = np.concatenate([res.results[i]["o"] for i in range(N_CORES)],
                           axis=1)
    return np.ascontiguousarray(out_t.T)
